# revision 4
# baseline (speedup 1.0000x reference)
"""Leaky-integrator (no spike) kernel for Trainium2.

Computes u[b, f, t] = tau_c[f] * u[b, f, t-1] + x[b, f, t] with u[.,.,-1] = 0,
tau_c = clip(tau, 0, 1), for x of shape (128, 1024, 500) fp32.

Strategy: data-parallel over batch (16 per core, 8 cores). Per core, the
F=1024 features are processed in 8 chunks of 128 (the SBUF partition dim);
the time recurrence runs along the free dim with the DVE's hardware scan
instruction (TensorTensorScanArith: state = data0*state + data1).
"""

import numpy as np

import concourse.bacc as bacc
import concourse.mybir as mybir
import concourse.tile as tile
from concourse.bass_utils import run_bass_kernel_spmd

B, F, T = 128, 1024, 500
N_CORES = 8
B_L = B // N_CORES          # 16 batches per core
P = 128                     # SBUF partitions
FC = F // P                 # 8 feature chunks per core

_BUILT = None


def build_bass():
    """Build the per-core Bass program (same program on all 8 cores)."""
    nc = bacc.Bacc("TRN2", target_bir_lowering=False, debug=False,
                   num_devices=N_CORES)
    f32 = mybir.dt.float32
    x_ap = nc.dram_tensor("x", [B_L, F, T], f32, kind="ExternalInput").ap()
    tau_ap = nc.dram_tensor("tau", [F], f32, kind="ExternalInput").ap()
    out_ap = nc.dram_tensor("out", [B_L, F, T], f32, kind="ExternalOutput").ap()

    with tile.TileContext(nc) as tc:
        with (
            tc.tile_pool(name="const", bufs=1) as const_pool,
            tc.tile_pool(name="io", bufs=3) as io_pool,
        ):
            # tau laid out [partition=f%128, chunk=f//128]
            tau_t = const_pool.tile([P, FC], f32)
            nc.sync.dma_start(out=tau_t[:], in_=tau_ap.rearrange("(c p) -> p c", p=P))

            # Broadcast each chunk's tau column along T once: bc_all[:, fc, :]
            ones = const_pool.tile([P, T], f32)
            nc.vector.memset(ones[:], 1.0)
            bc_all = const_pool.tile([P, FC, T], f32)
            for fc in range(FC):
                nc.vector.tensor_scalar_mul(
                    out=bc_all[:, fc, :], in0=ones[:], scalar1=tau_t[:, fc : fc + 1]
                )

            for fc in range(FC):
                sl = slice(fc * P, (fc + 1) * P)
                xin = io_pool.tile([P, B_L, T], f32)
                # DRAM x[:, sl, :] is [B_L, 128, T]; transpose view -> [128, B_L, T]
                nc.sync.dma_start(out=xin[:], in_=x_ap[:, sl, :].transpose([1, 0, 2]))
                for b in range(B_L):
                    nc.vector.tensor_tensor_scan(
                        out=xin[:, b, :],
                        data0=bc_all[:, fc, :],
                        data1=xin[:, b, :],
                        initial=0.0,
                        op0=mybir.AluOpType.mult,
                        op1=mybir.AluOpType.add,
                    )
                nc.sync.dma_start(
                    out=out_ap[:, sl, :].transpose([1, 0, 2]), in_=xin[:]
                )
    nc.compile()
    return nc


def _get_built():
    global _BUILT
    if _BUILT is None:
        _BUILT = build_bass()
    return _BUILT


def make_in_maps(x: np.ndarray, tau: np.ndarray) -> list[dict]:
    tau_c = np.clip(np.asarray(tau, dtype=np.float32), 0.0, 1.0)
    xs = np.asarray(x, dtype=np.float32)
    return [
        {"x": np.ascontiguousarray(xs[c * B_L : (c + 1) * B_L]), "tau": tau_c}
        for c in range(N_CORES)
    ]


def kernel(x: np.ndarray, tau: np.ndarray) -> np.ndarray:
    nc = _get_built()
    in_maps = make_in_maps(x, tau)
    res = run_bass_kernel_spmd(nc, in_maps, core_ids=list(range(N_CORES))).results
    return np.concatenate([res[c]["out"] for c in range(N_CORES)], axis=0)


# revision 6
# speedup vs baseline: 27.3182x; 27.3182x over previous
"""Leaky-integrator (no spike) kernel for Trainium2.

Computes u[b, f, t] = tau_c[f] * u[b, f, t-1] + x[b, f, t] with u[.,.,-1] = 0,
tau_c = clip(tau, 0, 1), for x of shape (128, 1024, 500) fp32.

Strategy: data-parallel over batch (16 per core, 8 cores). Per core, the
F=1024 features are processed in 8 chunks of 128 (the SBUF partition dim);
the time recurrence runs along the free dim with the DVE's hardware scan
instruction (TensorTensorScanArith: state = data0*state + data1).
"""

import numpy as np

import concourse.bacc as bacc
import concourse.mybir as mybir
import concourse.tile as tile
from concourse.bass_utils import run_bass_kernel_spmd

B, F, T = 128, 1024, 500
N_CORES = 8
B_L = B // N_CORES          # 16 batches per core
P = 128                     # SBUF partitions
FC = F // P                 # 8 feature chunks per core

_BUILT = None


def build_bass(repeat: int = 1):
    """Build the per-core Bass program (same program on all 8 cores).

    repeat > 1 re-runs the whole computation that many times inside one NEFF
    (same output; used by test.py to measure device time above the dispatch
    overhead of the axon tunnel).
    """
    nc = bacc.Bacc("TRN2", target_bir_lowering=False, debug=False,
                   num_devices=N_CORES)
    f32 = mybir.dt.float32
    x_ap = nc.dram_tensor("x", [B_L, F, T], f32, kind="ExternalInput").ap()
    tau_ap = nc.dram_tensor("tau", [F], f32, kind="ExternalInput").ap()
    out_ap = nc.dram_tensor("out", [B_L, F, T], f32, kind="ExternalOutput").ap()

    with tile.TileContext(nc) as tc:
        with (
            tc.tile_pool(name="const", bufs=1) as const_pool,
            tc.tile_pool(name="io", bufs=3) as io_pool,
        ):
            # tau laid out [partition=f%128, chunk=f//128]
            tau_t = const_pool.tile([P, FC], f32)
            nc.sync.dma_start(out=tau_t[:], in_=tau_ap.rearrange("(c p) -> p c", p=P))

            # Broadcast each chunk's tau column along T once: bc_all[:, fc, :]
            ones = const_pool.tile([P, T], f32)
            nc.vector.memset(ones[:], 1.0)
            bc_all = const_pool.tile([P, FC, T], f32)
            for fc in range(FC):
                nc.vector.tensor_scalar_mul(
                    out=bc_all[:, fc, :], in0=ones[:], scalar1=tau_t[:, fc : fc + 1]
                )

            for _rep in range(repeat):
              for fc in range(FC):
                sl = slice(fc * P, (fc + 1) * P)
                xin = io_pool.tile([P, B_L, T], f32)
                # DRAM x[:, sl, :] is [B_L, 128, T]; transpose view -> [128, B_L, T]
                nc.sync.dma_start(out=xin[:], in_=x_ap[:, sl, :].transpose([1, 0, 2]))
                for b in range(B_L):
                    nc.vector.tensor_tensor_scan(
                        out=xin[:, b, :],
                        data0=bc_all[:, fc, :],
                        data1=xin[:, b, :],
                        initial=0.0,
                        op0=mybir.AluOpType.mult,
                        op1=mybir.AluOpType.add,
                    )
                nc.sync.dma_start(
                    out=out_ap[:, sl, :].transpose([1, 0, 2]), in_=xin[:]
                )
    nc.compile()
    return nc


def _get_built():
    global _BUILT
    if _BUILT is None:
        _BUILT = build_bass()
    return _BUILT


def make_in_maps(x: np.ndarray, tau: np.ndarray) -> list[dict]:
    tau_c = np.clip(np.asarray(tau, dtype=np.float32), 0.0, 1.0)
    xs = np.asarray(x, dtype=np.float32)
    return [
        {"x": np.ascontiguousarray(xs[c * B_L : (c + 1) * B_L]), "tau": tau_c}
        for c in range(N_CORES)
    ]


def kernel(x: np.ndarray, tau: np.ndarray) -> np.ndarray:
    nc = _get_built()
    in_maps = make_in_maps(x, tau)
    res = run_bass_kernel_spmd(nc, in_maps, core_ids=list(range(N_CORES))).results
    return np.concatenate([res[c]["out"] for c in range(N_CORES)], axis=0)


# revision 7
# speedup vs baseline: 45.8729x; 1.6792x over previous
"""Leaky-integrator (no spike) kernel for Trainium2.

Computes u[b, f, t] = tau_c[f] * u[b, f, t-1] + x[b, f, t] with u[.,.,-1] = 0,
tau_c = clip(tau, 0, 1), for x of shape (128, 1024, 500) fp32.

Strategy: data-parallel over batch (16 per core, 8 cores). Per core, the
F=1024 features are processed in 8 chunks of 128 (the SBUF partition dim);
the time recurrence runs along the free dim with the DVE's hardware scan
instruction (TensorTensorScanArith: state = data0*state + data1).
"""

import numpy as np

import concourse.bacc as bacc
import concourse.mybir as mybir
import concourse.tile as tile
from concourse.bass_utils import run_bass_kernel_spmd

B, F, T = 128, 1024, 500
N_CORES = 8
B_L = B // N_CORES          # 16 batches per core
P = 128                     # SBUF partitions
FC = F // P                 # 8 feature chunks per core

_BUILT = None


def build_bass(repeat: int = 1):
    """Build the per-core Bass program (same program on all 8 cores).

    repeat > 1 re-runs the whole computation that many times inside one NEFF
    (same output; used by test.py to measure device time above the dispatch
    overhead of the axon tunnel).
    """
    nc = bacc.Bacc("TRN2", target_bir_lowering=False, debug=False,
                   num_devices=N_CORES)
    f32 = mybir.dt.float32
    x_ap = nc.dram_tensor("x", [B_L, F, T], f32, kind="ExternalInput").ap()
    tau_ap = nc.dram_tensor("tau", [F], f32, kind="ExternalInput").ap()
    out_ap = nc.dram_tensor("out", [B_L, F, T], f32, kind="ExternalOutput").ap()

    with tile.TileContext(nc) as tc:
        with (
            tc.tile_pool(name="const", bufs=1) as const_pool,
            tc.tile_pool(name="io", bufs=3) as io_pool,
        ):
            # tau laid out [partition=f%128, chunk=f//128]
            tau_t = const_pool.tile([P, FC], f32)
            nc.sync.dma_start(out=tau_t[:], in_=tau_ap.rearrange("(c p) -> p c", p=P))

            # Broadcast each chunk's tau column along T once: bc_all[:, fc, :]
            ones = const_pool.tile([P, T], f32)
            nc.vector.memset(ones[:], 1.0)
            bc_all = const_pool.tile([P, FC, T], f32)
            for fc in range(FC):
                nc.vector.tensor_scalar_mul(
                    out=bc_all[:, fc, :], in0=ones[:], scalar1=tau_t[:, fc : fc + 1]
                )

            for _rep in range(repeat):
              for fc in range(FC):
                sl = slice(fc * P, (fc + 1) * P)
                xin = io_pool.tile([P, B_L, T], f32)
                # DRAM x[:, sl, :] is [B_L, 128, T]; transpose view -> [128, B_L, T]
                nc.sync.dma_start(out=xin[:], in_=x_ap[:, sl, :].transpose([1, 0, 2]))
                for b in range(B_L):
                    nc.vector.tensor_tensor_scan(
                        out=xin[:, b, :],
                        data0=bc_all[:, fc, :],
                        data1=xin[:, b, :],
                        initial=0.0,
                        op0=mybir.AluOpType.mult,
                        op1=mybir.AluOpType.add,
                    )
                # Output DMAs ride the Activation HWDGE ring so the input
                # (SP ring) and output streams use both hardware DGE rings.
                nc.scalar.dma_start(
                    out=out_ap[:, sl, :].transpose([1, 0, 2]), in_=xin[:]
                )
    nc.compile()
    return nc


def _get_built():
    global _BUILT
    if _BUILT is None:
        _BUILT = build_bass()
    return _BUILT


def make_in_maps(x: np.ndarray, tau: np.ndarray) -> list[dict]:
    tau_c = np.clip(np.asarray(tau, dtype=np.float32), 0.0, 1.0)
    xs = np.asarray(x, dtype=np.float32)
    return [
        {"x": np.ascontiguousarray(xs[c * B_L : (c + 1) * B_L]), "tau": tau_c}
        for c in range(N_CORES)
    ]


def kernel(x: np.ndarray, tau: np.ndarray) -> np.ndarray:
    nc = _get_built()
    in_maps = make_in_maps(x, tau)
    res = run_bass_kernel_spmd(nc, in_maps, core_ids=list(range(N_CORES))).results
    return np.concatenate([res[c]["out"] for c in range(N_CORES)], axis=0)


# revision 9
# speedup vs baseline: 60.3419x; 1.3154x over previous
"""Leaky-integrator (no spike) kernel for Trainium2.

Computes u[b, f, t] = tau_c[f] * u[b, f, t-1] + x[b, f, t] with u[.,.,-1] = 0,
tau_c = clip(tau, 0, 1), for x of shape (128, 1024, 500) fp32.

Strategy: data-parallel over batch (16 per core, 8 cores). Per core, the
F=1024 features are processed in 8 chunks of 128 (the SBUF partition dim);
the time recurrence runs along the free dim with the DVE's hardware scan
instruction (TensorTensorScanArith: state = data0*state + data1).
"""

import numpy as np

import concourse.bacc as bacc
import concourse.mybir as mybir
import concourse.tile as tile
from concourse.bass_utils import run_bass_kernel_spmd

B, F, T = 128, 1024, 500
N_CORES = 8
B_L = B // N_CORES          # 16 batches per core
P = 128                     # SBUF partitions
FC = F // P                 # 8 feature chunks per core

_BUILT = None


def build_bass(repeat: int = 1):
    """Build the per-core Bass program (same program on all 8 cores).

    repeat > 1 re-runs the whole computation that many times inside one NEFF
    (same output; used by test.py to measure device time above the dispatch
    overhead of the axon tunnel).
    """
    nc = bacc.Bacc("TRN2", target_bir_lowering=False, debug=False,
                   num_devices=N_CORES)
    f32 = mybir.dt.float32
    x_ap = nc.dram_tensor("x", [B_L, F, T], f32, kind="ExternalInput").ap()
    tau_ap = nc.dram_tensor("tau", [F], f32, kind="ExternalInput").ap()
    out_ap = nc.dram_tensor("out", [B_L, F, T], f32, kind="ExternalOutput").ap()

    with tile.TileContext(nc) as tc:
        with (
            tc.tile_pool(name="const", bufs=1) as const_pool,
            tc.tile_pool(name="io", bufs=4) as io_pool,
        ):
            # tau laid out [partition=f%128, chunk=f//128]
            tau_t = const_pool.tile([P, FC], f32)
            nc.sync.dma_start(out=tau_t[:], in_=tau_ap.rearrange("(c p) -> p c", p=P))

            # Broadcast each chunk's tau column along T once: bc_all[:, fc, :]
            ones = const_pool.tile([P, T], f32)
            nc.vector.memset(ones[:], 1.0)
            bc_all = const_pool.tile([P, FC, T], f32)
            for fc in range(FC):
                nc.vector.tensor_scalar_mul(
                    out=bc_all[:, fc, :], in0=ones[:], scalar1=tau_t[:, fc : fc + 1]
                )

            # Input DMAs ride the SP HWDGE ring, output DMAs the Activation
            # ring, and each chunk's transfer is split into 4 x 1MB so scans
            # start before the whole chunk lands and more queue lanes fill.
            SPLIT, BS = 4, B_L // 4
            for _rep in range(repeat):
              for fc in range(FC):
                sl = slice(fc * P, (fc + 1) * P)
                xin = io_pool.tile([P, B_L, T], f32)
                for s in range(SPLIT):
                    bsl = slice(s * BS, (s + 1) * BS)
                    # DRAM x[bsl, sl, :] is [BS, 128, T]; transpose -> [128, BS, T]
                    nc.sync.dma_start(
                        out=xin[:, bsl, :],
                        in_=x_ap[bsl, sl, :].transpose([1, 0, 2]),
                    )
                for b in range(B_L):
                    nc.vector.tensor_tensor_scan(
                        out=xin[:, b, :],
                        data0=bc_all[:, fc, :],
                        data1=xin[:, b, :],
                        initial=0.0,
                        op0=mybir.AluOpType.mult,
                        op1=mybir.AluOpType.add,
                    )
                for s in range(SPLIT):
                    bsl = slice(s * BS, (s + 1) * BS)
                    nc.scalar.dma_start(
                        out=out_ap[bsl, sl, :].transpose([1, 0, 2]),
                        in_=xin[:, bsl, :],
                    )
    nc.compile()
    return nc


def _get_built():
    global _BUILT
    if _BUILT is None:
        _BUILT = build_bass()
    return _BUILT


def make_in_maps(x: np.ndarray, tau: np.ndarray) -> list[dict]:
    tau_c = np.clip(np.asarray(tau, dtype=np.float32), 0.0, 1.0)
    xs = np.asarray(x, dtype=np.float32)
    return [
        {"x": np.ascontiguousarray(xs[c * B_L : (c + 1) * B_L]), "tau": tau_c}
        for c in range(N_CORES)
    ]


def kernel(x: np.ndarray, tau: np.ndarray) -> np.ndarray:
    nc = _get_built()
    in_maps = make_in_maps(x, tau)
    res = run_bass_kernel_spmd(nc, in_maps, core_ids=list(range(N_CORES))).results
    return np.concatenate([res[c]["out"] for c in range(N_CORES)], axis=0)
